# revision 28
# baseline (speedup 1.0000x reference)
"""Deformable Conv1d kernel for 8 Trainium2 NeuronCores.

Problem (hardcoded shapes):
  x      [8, 512, 4096] f32
  w_off  [6, 512, 3]    f32   (offset-prediction conv weights; only even channels used)
  b_off  [6]            f32
  w_conv [512, 1536, 1] f32   (1x1 conv over the C*K "scrambled" im2col view)
  b_conv [512]          f32
  out    [8, 512, 4096] f32

Sharding: pure data-parallel over batch N=8 -> one sample per NeuronCore.

Math (faithful to the reference's raw .reshape view):
  out[n, o, 512*b + c] = sum_{i} W[o, i] * G_b[i, c] + b_conv[o]
  where i = k*512 + m,  G_b[i, c] = x_deform[n, c, l=8m+b, k]
  x_deform[., c, l, k] = (1-a)*x_pad[c, li] + a*x_pad[c, ri]
  grid = clip(l + 1 + off[k, l], 0, 4097), li = floor(grid), ri = min(li+1, 4097)
  off[k, l] = offset-conv output channel 2k.

Split: the bilinear gather (offset conv + interp, ~0.1% of the FLOPs) runs
on host (on-device SWDGE gathers crash this environment's runtime); the
device does the 51.5 GFLOP GEMM, one sample per core, in bf16.

Device-side schedule (built for the TRN2 timing model):
  - gmat/wt/out in bf16: 1 PE cycle/row (fp32 is 4) and half the DMA.
  - wt is interleaved with block 0 of gmat in ONE DRAM tensor ("wg") so
    each contraction chunk (weights + data) lands in a single DMA --
    per-DMA HWDGE overhead (625ns) otherwise throttles the head of the
    stream below the PE's consumption rate.
  - warm-up matmuls on scratch SBUF keep the PE busy (and its p-state
    ramp running) while the first real chunks are still in flight.
  - all DMAs on the SP queue: loads first, in PE consumption order with
    granularity matched to the consumption rate; stores queue up behind
    them and drain once the load stream ends.
  - PSUM accumulates f32 across the 12 k-chunks; bias-add on DVE; the
    last block runs oc-outer (final group split across two PSUM banks)
    so its bias+stores drain under the PE.
"""

import numpy as np

C = 512
L = 4096
K = 3
LP = L + 2          # padded length 4098
CC = 4              # out-channel chunks of 128
B = 8               # output column blocks (j = 512*b + c)
G = 12              # contraction chunks of 128 (1536 = 12*128)
P = 128
N_WARM = 96         # warm-up matmuls before the first data-dependent one
WARM_F = 32         # free dim of each warm-up matmul

_PROGRAM_CACHE = {}


def _build_gemm_program(dt_name="bf16"):
    """GEMM-only program: host supplies the interpolated im2col matrices.

    dt_name: dtype of gmat/wt/out and the matmul ('bf16' | 'f32' | 'f32r').
    """
    import concourse.mybir as mybir
    import concourse.tile as tile
    from concourse import bacc

    f32 = mybir.dt.float32
    if dt_name == "bf16":
        dt, mm_cast = mybir.dt.bfloat16, None
    elif dt_name == "f32r":
        dt, mm_cast = f32, mybir.dt.float32r
    else:
        dt, mm_cast = f32, None

    nc = bacc.Bacc(num_swdge_queues=1)
    # wg rows: for g in 0..11: [wt_g (128); gmat_block0_g (128)], then
    # gmat blocks 1..7 (12*128 rows each)
    wg_in = nc.declare_dram_parameter(
        "wg", [(2 * G + (B - 1) * G) * P, C], dt, isOutput=False)
    bconv_in = nc.declare_dram_parameter("bconv", [P, CC], f32, isOutput=False)
    out_d = nc.declare_dram_parameter("out", [C, L], dt, isOutput=True)

    with tile.TileContext(nc) as tc:
        with tc.tile_pool(name="const", bufs=1) as const, \
             tc.tile_pool(name="pso", bufs=2, space="PSUM") as pso, \
             tc.tile_pool(name="ost", bufs=12) as ostp:
            # wtgl[p, g*2C + c2]: c2 in [0,C) = wt chunk g, [C,2C) = block-0
            # gmat chunk g
            wtgl = const.tile([P, 2 * G * C], dt)
            glall = const.tile([P, (B - 1) * G * C], dt)  # blocks 1..7
            bconv_sb = const.tile([P, CC], f32)
            scratch = const.tile([P, WARM_F], dt)  # warm-up operand

            def load_pair(g):
                nc.sync.dma_start(
                    out=wtgl[:, g * 2 * C:(g + 1) * 2 * C].rearrange(
                        "p (r c) -> p r c", r=2),
                    in_=wg_in[g * 2 * P:(g + 1) * 2 * P, :].rearrange(
                        "(r p) c -> p r c", r=2, p=P),
                )

            def load_gl(b, g0, g1):
                n = g1 - g0
                r0 = 2 * G * P + (b - 1) * G * P
                o0 = (b - 1) * G * C
                nc.sync.dma_start(
                    out=glall[:, o0 + g0 * C:o0 + g1 * C].rearrange(
                        "p (g c) -> p g c", g=n),
                    in_=wg_in[r0 + g0 * P:r0 + g1 * P, :].rearrange(
                        "(g p) c -> p g c", g=n, p=P),
                )

            for g in range(G):
                load_pair(g)
            nc.sync.dma_start(out=bconv_sb[:], in_=bconv_in[:])
            load_gl(1, 0, 3)
            load_gl(1, 3, 6)
            load_gl(1, 6, 12)
            for b in range(2, B):
                load_gl(b, 0, G)

            def mm(b, g, oc, out_ap, cs=None):
                lhsT = wtgl[:, g * 2 * C + oc * P:g * 2 * C + (oc + 1) * P]
                if b == 0:
                    rhs = wtgl[:, g * 2 * C + C:g * 2 * C + 2 * C]
                else:
                    o0 = (b - 1) * G * C
                    rhs = glall[:, o0 + g * C:o0 + (g + 1) * C]
                if cs is not None:
                    rhs = rhs[:, cs]
                if mm_cast is not None:
                    lhsT = lhsT.bitcast(mm_cast)
                    rhs = rhs.bitcast(mm_cast)
                nc.tensor.matmul(
                    out=out_ap, lhsT=lhsT, rhs=rhs,
                    start=(g == 0), stop=(g == G - 1),
                )

            def bias_store(b, oc, ps):
                ot = ostp.tile([P, 512], dt, tag="ostage", name="ot")
                nc.vector.tensor_scalar(
                    out=ot[:], in0=ps[:], scalar1=bconv_sb[:, oc:oc + 1],
                    scalar2=None, op0=mybir.AluOpType.add,
                )
                nc.sync.dma_start(
                    out=out_d[oc * P:(oc + 1) * P, b * 512:(b + 1) * 512],
                    in_=ot[:],
                )

            # warm-up: keeps the PE busy (and its p-state ramp running)
            # while the first real chunks are in flight; results unread
            if N_WARM:
                nc.vector.memset(scratch[:], 0)
                psw = pso.tile([P, 512], f32, tag="ps0", name="psw")
                sc = scratch[:]
                if mm_cast is not None:
                    sc = sc.bitcast(mm_cast)
                for _ in range(N_WARM):
                    nc.tensor.matmul(
                        out=psw[0:WARM_F, 0:WARM_F], lhsT=sc, rhs=sc,
                        start=True, stop=True,
                    )

            for b in range(B):
                ps = [
                    pso.tile([P, 512], f32, tag=f"ps{oc}", name=f"ps{oc}")
                    for oc in range(CC)
                ]
                if b < B - 1:
                    # g-outer: streams behind the loads at chunk granularity
                    for g in range(G):
                        for oc in range(CC):
                            mm(b, g, oc, ps[oc][:])
                    for oc in range(CC):
                        bias_store(b, oc, ps[oc])
                else:
                    # last block oc-outer: bias+store per oc drain under
                    # the PE while later oc groups still stream; the very
                    # last group is split into column halves so the final
                    # dependent bias+store chain is half-sized
                    for oc in range(CC - 1):
                        for g in range(G):
                            mm(b, g, oc, ps[oc][:])
                        bias_store(b, oc, ps[oc])
                    oc = CC - 1
                    for c0, c1 in ((0, 256), (256, 512)):
                        cs = slice(c0, c1)
                        if c0 == 0:
                            pst = ps[oc]
                        else:
                            # fresh tile from the ps0 rotation -> different
                            # PSUM bank, so this group's writes don't wait
                            # for the first half's bias to drain the bank
                            pst = pso.tile([P, 512], f32, tag="ps0",
                                           name="psB")
                        for g in range(G):
                            mm(b, g, oc, pst[:, cs], cs=cs)
                        ot = ostp.tile([P, c1 - c0], dt, tag=f"osth{c0}",
                                       name="oth")
                        nc.vector.tensor_scalar(
                            out=ot[:], in0=pst[:, cs],
                            scalar1=bconv_sb[:, oc:oc + 1],
                            scalar2=None, op0=mybir.AluOpType.add,
                        )
                        nc.sync.dma_start(
                            out=out_d[oc * P:(oc + 1) * P,
                                      b * 512 + c0:b * 512 + c1],
                            in_=ot[:],
                        )
    nc.finalize()
    return nc


def _host_gather(x, w_off, b_off):
    """offset conv + bilinear gather on host -> im2col mats [N, B*G*P, C]."""
    N = x.shape[0]
    w_sel = w_off[[0, 2, 4]].astype(np.float32)      # [3, 512, 3]
    b_sel = b_off[[0, 2, 4]].astype(np.float32)
    base = np.arange(L, dtype=np.float32) + 1.0
    i_idx = np.arange(G * P)
    jj = i_idx // 512                                 # tap k per row
    m = i_idx % 512
    # l_mat[b, i] = 8*m[i] + b
    l_mat = (8 * m)[None, :] + np.arange(B)[:, None]  # [B, G*P] int
    jj_mat = np.broadcast_to(jj[None, :], l_mat.shape)
    gmats = np.empty((N, B * G * P, C), np.float32)
    for n in range(N):
        xs = x[n].astype(np.float32)
        x_pad = np.zeros((C, LP), np.float32)
        x_pad[:, 1:LP - 1] = xs
        off = b_sel[:, None] + sum(
            w_sel[:, :, t] @ x_pad[:, t:t + L] for t in range(K))  # [3, L]
        grid = np.clip(base[None, :] + off, 0.0, float(LP - 1))
        li = np.floor(grid)
        alpha = (grid - li).astype(np.float32)
        ri = np.minimum(li + 1.0, float(LP - 1)).astype(np.int32)
        li = li.astype(np.int32)
        xpt = np.zeros((LP, C), np.float32)
        xpt[1:LP - 1] = xs.T
        a = alpha[jj_mat, l_mat].reshape(-1, 1)       # [B*G*P, 1]
        lif = li[jj_mat, l_mat].reshape(-1)
        rif = ri[jj_mat, l_mat].reshape(-1)
        gmats[n] = (1.0 - a) * xpt[lif] + a * xpt[rif]
    return gmats


def run(x, w_off, b_off, w_conv, b_conv, mm_dt="bf16", tb_dt=None, trace=False):
    from concourse.bass_utils import run_bass_kernel_spmd

    dt_name = mm_dt if mm_dt in ("bf16", "f32", "f32r") else "bf16"
    key = ("gemm", dt_name)
    if key not in _PROGRAM_CACHE:
        _PROGRAM_CACHE[key] = _build_gemm_program(dt_name)
    nc = _PROGRAM_CACHE[key]

    wt = np.ascontiguousarray(w_conv[:, :, 0].T.astype(np.float32))  # [1536, 512]
    bconv = np.ascontiguousarray(
        b_conv.reshape(CC, P).T).astype(np.float32)   # [128, 4]
    gmats = _host_gather(x, w_off, b_off)             # [N, B*G*P, C] f32
    if dt_name == "bf16":
        import ml_dtypes
        wt = wt.astype(ml_dtypes.bfloat16)
        gmats = gmats.astype(ml_dtypes.bfloat16)
    wtr = wt.reshape(G, P, C)
    in_maps = []
    for n in range(x.shape[0]):
        head = np.stack([wtr, gmats[n][:G * P].reshape(G, P, C)], axis=1)
        wg = np.concatenate(
            [head.reshape(2 * G * P, C), gmats[n][G * P:]], axis=0)
        in_maps.append({"wg": np.ascontiguousarray(wg), "bconv": bconv})
    res = run_bass_kernel_spmd(nc, in_maps, list(range(len(in_maps))), trace=False)
    out = np.stack([r["out"] for r in res.results], axis=0).astype(np.float32)
    return out, res


def kernel(x, w_off, b_off, w_conv, b_conv):
    out, _ = run(
        np.asarray(x), np.asarray(w_off), np.asarray(b_off), np.asarray(w_conv),
        np.asarray(b_conv), mm_dt="bf16",
    )
    return out
